# revision 1
# baseline (speedup 1.0000x reference)
"""Fused DDPM dynamic-conv kernel for TRN2 (8 NeuronCores), v2.

Math (reference):
  kernels = einsum('nchw,oc->nohw', y, gen_w) + gen_b        # o = d*576 + c*9 + t
  r_d     = sum_t kernels[d,c,t] * shift(x, tap t, dil d)    # d in {1,3,5}
  out     = conv3x3([x, r1, r3, r5], fuse_w) + fuse_b

Sharding: 8 cores = 4 batches x 2 H-halves (48 output rows each).
Per core, the 50 kern rows (48 + 1 halo each side) are split into two
26-row blocks (2-row overlap) packed on SBUF partitions: p = 64*blk + c.
All tap shifts are free-dim AP offsets into a zero-padded x tile.

v2 vs v1: all matmul operands are bf16 (same 1 cyc/row as f32r in the
cost model, half the DMA/SBUF), and the tap accumulation no longer runs
on the PE as identity matmuls (saving ~28us of PE time).  Each tap
product lands in SBUF as bf16; per (d, chunk) the 9 products reduce via
a tree whose first level rides the otherwise-idle DMA fabric as two
2-plane accumulating DMAs (gpsimd software-DGE cce add; transfers above
~2 planes/partition accumulate incorrectly on hardware, so they stay
2-plane), and whose remaining levels are bf16 tensor_tensor adds on DVE
in the 2x packed mode.  The 27 per-tap kern planes leave PSUM via two
routes only (GPSIMD cannot touch PSUM and TensorScalarPtr is not a
legal Pool opcode):
  ACT taps (6/9): ACT casts PSUM->bf16 with the gen_b bias fused
            (activation Identity + bias); the bf16 multiply then runs
            on DVE (2x mode) or GPSIMD tensor_tensor (1 tap).
  DVE taps (3/9, interleaved at slots 2/5/8): one scalar_tensor_tensor
            (kp + gb) * x straight from PSUM.
The fuse conv is 9 taps x 4 groups of block-diag K=128 bf16 matmuls
accumulating in PSUM; fuse_b is added by the ACT Identity copy out.
Fuse work is queued per 5-row out-chunk as racc rows complete and
pumped into the PE stream with a 3-group lag so the in-order PE never
stalls on a racc semaphore; the last two out-chunks are small (3+1
rows) so the unavoidable post-loop fuse tail is short.
"""

import numpy as np

K = 3
NB, C, H, W = 4, 64, 96, 96
NCORES = 8
HH = 48            # output rows per core
BLK = 26           # kern rows per block (24 out + 1 halo + 1 overlap)
XR = BLK + 10      # x rows per block (halo 5 each side for dil 5)
WP = W + 10        # padded width for x
RW = W + 2         # padded width for racc
DILS = (1, 3, 5)
_CHUNK_VARIANTS = {
    "a": ((0, 10), (10, 10), (20, 4), (24, 2)),
    "b": ((0, 10), (10, 8), (18, 6), (24, 2)),
    "c": ((0, 6), (6, 10), (16, 8), (24, 2)),
    "d": ((0, 8), (8, 10), (18, 6), (24, 2)),
}
import os
CHUNKS = _CHUNK_VARIANTS[os.environ.get("K_CHUNKS", "a")]  # kern-row chunks
FCHUNKS = ((1, 5), (6, 5), (11, 5), (16, 5), (21, 3), (24, 1))  # out-row chunks
import os
PUMP_N = int(os.environ.get("K_PUMP_N", "2"))
FUSE_PARTS = int(os.environ.get("K_FUSE_PARTS", "9"))
XFIRST = os.environ.get("K_XFIRST", "1") == "1"
TAP_PUMP = int(os.environ.get("K_TAP_PUMP", "0"))   # fuse part every N taps (0=off)
PPOOL = int(os.environ.get("K_PPOOL", "3"))
KSPOOL = int(os.environ.get("K_KSPOOL", "6"))
PUMP_LAG = int(os.environ.get("K_PUMP_LAG", "2"))  # groups between queue and pump
PUMP_LAG_LATE = int(os.environ.get("K_PUMP_LAG_LATE", "3"))  # lag for last fchunks
LATE_N = int(os.environ.get("K_LATE_N", "2"))  # how many final fchunks are "late"

# per-tap product routing (tap index 0..8).  GPSIMD cannot touch PSUM and
# DMA cannot read PSUM, so kern planes leave PSUM only via ACT (cast+bias
# to bf16) or via DVE scalar_tensor_tensor directly.  The bf16 multiplies
# for ACT-routed taps are split between Pool (SBUF-only is fine there) and
# DVE.  The first two levels of the accumulation tree ride the otherwise
# idle DMA fabric as accumulating SBUF->SBUF DMAs (software-DGE cce add).
_NACT = int(os.environ.get("K_NACT", "7"))
_NPM = int(os.environ.get("K_NPOOLM", "0"))
_IL = int(os.environ.get("K_INTERLEAVE", "1"))
if _IL:
    _DVE_TAPS = {9 - 2 * i - 1 for i in range(9 - _NACT)}  # spread-ish: 8,6,4...
    _pos = {"a": (2, 5, 8, 0, 4), "b": (3, 6, 8, 0, 4), "c": (4, 7, 8, 0, 4),
            "d": (1, 5, 8, 0, 4)}[os.environ.get("K_DPOS", "c")]
    _DVE_TAPS = set(list(_pos)[:9 - _NACT])
else:
    _DVE_TAPS = set(range(_NACT, 9))
ACT_TAPS = tuple(t for t in range(9) if t not in _DVE_TAPS)
POOL_MULT = ACT_TAPS[:_NPM]      # ACT-taps whose multiply runs on Pool (STT)
DMA_ACC = os.environ.get("K_DMA_ACC", "1") == "1"
DMA_L1 = os.environ.get("K_DMA_L1", "0") == "1"

_built = None


def _build():
    import concourse.mybir as mybir
    from concourse import bacc
    from concourse.tile import TileContext

    f32 = mybir.dt.float32
    bf16 = mybir.dt.bfloat16
    add = mybir.AluOpType.add
    mult = mybir.AluOpType.mult
    ident = mybir.ActivationFunctionType.Identity

    nc = bacc.Bacc()
    xh = nc.dram_tensor("xh", [C, 60, WP], bf16, kind="ExternalInput")
    yh = nc.dram_tensor("yh", [C, 50, W], bf16, kind="ExternalInput")
    wg = nc.dram_tensor("wg", [128, 27 * 128], bf16, kind="ExternalInput")
    gb = nc.dram_tensor("gb", [128, 27], f32, kind="ExternalInput")
    fw = nc.dram_tensor("fw", [128, 9 * 4 * 128], bf16, kind="ExternalInput")
    fb = nc.dram_tensor("fb", [128, 1], f32, kind="ExternalInput")
    rm = nc.dram_tensor("rm", [128, 2], f32, kind="ExternalInput")
    out = nc.dram_tensor("out", [C, HH, W], f32, kind="ExternalOutput")

    with TileContext(nc) as tc:
        with (
            tc.tile_pool(name="const", bufs=1) as cpool,
            tc.tile_pool(name="prod", bufs=PPOOL) as ppool,
            tc.tile_pool(name="ksb", bufs=KSPOOL) as kspool,
            tc.tile_pool(name="kpsum", bufs=3, space="PSUM") as kpool,
            tc.tile_pool(name="fpsum", bufs=2, space="PSUM") as fpool,
        ):
            xpad = cpool.tile([128, XR, WP], bf16)
            ysbc = [cpool.tile([128, nrc * W], bf16, tag=f"ysb{ci}",
                               name=f"ysb{ci}")
                    for ci, (r0, nrc) in enumerate(CHUNKS)]
            wgd = [cpool.tile([128, 9 * 128], bf16, tag=f"wg{dd}",
                              name=f"wg{dd}") for dd in range(3)]
            gbsb = cpool.tile([128, 27], f32)
            fwsb = cpool.tile([128, 9 * 4 * 128], bf16)
            fbsb = cpool.tile([128, 1], f32)
            rmsb = cpool.tile([128, 2], f32)
            racc = cpool.tile([128, 3, BLK, RW], bf16)
            osb = cpool.tile([128, 24, W], f32)

            # zero the 1-col borders of racc (cols 0 and 97)
            nc.vector.memset(racc[:, :, :, 0:RW:RW - 1], 0.0)
            # load order matters: chunk-0/d-1 work needs only ysb chunk 0,
            # wg d-block 0, gb and the top of xpad; fuse weights much later.
            def ysv(ci):
                return ysbc[ci][:].rearrange("p (r w) -> p r w",
                                             r=CHUNKS[ci][1])
            def yload(ci, split=0):
                r0, nrc = CHUNKS[ci]
                if split:
                    nb0 = nrc // 2
                    nc.sync.dma_start(out=ysv(ci)[0:64, 0:nb0],
                                      in_=yh[:, r0:r0 + nb0, :])
                    nc.sync.dma_start(out=ysv(ci)[64:128, 0:nb0],
                                      in_=yh[:, 24 + r0:24 + r0 + nb0, :])
                    nc.sync.dma_start(out=ysv(ci)[0:64, nb0:nrc],
                                      in_=yh[:, r0 + nb0:r0 + nrc, :])
                    nc.sync.dma_start(out=ysv(ci)[64:128, nb0:nrc],
                                      in_=yh[:, 24 + r0 + nb0:24 + r0 + nrc, :])
                else:
                    nc.sync.dma_start(out=ysv(ci)[0:64],
                                      in_=yh[:, r0:r0 + nrc, :])
                    eng2 = (nc.scalar if (ci == 0 and os.environ.get(
                        "K_Y0ACT", "0") == "1") else nc.sync)
                    eng2.dma_start(out=ysv(ci)[64:128],
                                   in_=yh[:, 24 + r0:24 + r0 + nrc, :])
            yload(0)
            nc.sync.dma_start(out=wgd[0][:, :], in_=wg[:, 0:1152])
            nc.sync.dma_start(out=gbsb[:, :], in_=gb[:, :])
            nc.sync.dma_start(out=xpad[0:64, 0:21, :], in_=xh[:, 0:21, :])
            nc.sync.dma_start(out=xpad[64:128, 0:21, :], in_=xh[:, 24:45, :])
            nc.sync.dma_start(out=xpad[0:64, 21:XR, :], in_=xh[:, 21:XR, :])
            nc.sync.dma_start(out=xpad[64:128, 21:XR, :],
                              in_=xh[:, 45:24 + XR, :])
            nc.sync.dma_start(out=wgd[1][:, :], in_=wg[:, 1152:2304])
            yload(1)
            nc.sync.dma_start(out=wgd[2][:, :], in_=wg[:, 2304:3456])
            yload(2)
            yload(3)
            nc.sync.dma_start(out=rmsb[:, :], in_=rm[:, :])
            nc.sync.dma_start(out=fwsb[:, 0:2304], in_=fw[:, 0:2304])
            nc.sync.dma_start(out=fwsb[:, 2304:], in_=fw[:, 2304:])
            nc.sync.dma_start(out=fbsb[:, :], in_=fb[:, :])

            fuse_state = {}

            def fuse_unit(o0, nr, part):
                # 12 of the 36 accumulating matmuls for one out-row chunk;
                # part 0 allocates the psum tile, last part copies out via ACT
                if part == 0:
                    fp = fpool.tile([128, 5, W], f32, tag="fp")
                    fuse_state[o0] = fp
                fp = fuse_state[o0]
                ps = fp[:, 0:nr, :]
                if XFIRST:
                    items = [(di, dj, 0) for di in (-1, 0, 1)
                             for dj in (-1, 0, 1)]
                    items += [(di, dj, g) for g in (1, 2, 3)
                              for di in (-1, 0, 1) for dj in (-1, 0, 1)]
                else:
                    items = [(di, dj, g) for di in (-1, 0, 1)
                             for dj in (-1, 0, 1) for g in range(4)]
                usz = 36 // FUSE_PARTS
                for cnt in range(part * usz, part * usz + usz):
                    di, dj, g = items[cnt]
                    ij = (di + 1) * 3 + (dj + 1)
                    if g == 0:
                        rhs = xpad[:, o0 + di + 5:o0 + di + 5 + nr,
                                   5 + dj:5 + dj + W]
                    else:
                        rhs = racc[:, g - 1, o0 + di:o0 + di + nr,
                                   1 + dj:1 + dj + W]
                    nc.tensor.matmul(
                        ps, fwsb[:, (ij * 4 + g) * 128:(ij * 4 + g + 1) * 128],
                        rhs, start=(cnt == 0), stop=(cnt == 35),
                    )
                if part == FUSE_PARTS - 1:
                    nc.scalar.activation(
                        osb[:, o0 - 1:o0 - 1 + nr, :], ps, ident,
                        bias=fbsb[:, 0:1])
                    del fuse_state[o0]
                    nc.sync.dma_start(out=out[:, o0 - 1:o0 - 1 + nr, :],
                                      in_=osb[0:64, o0 - 1:o0 - 1 + nr, :])
                    nc.sync.dma_start(out=out[:, 23 + o0:23 + o0 + nr, :],
                                      in_=osb[64:128, o0 - 1:o0 - 1 + nr, :])

            fuse_q = []

            pumped_parts = {}   # fchunk idx -> #parts already pumped

            def pump_fuse(maxn, group=None):
                n = 0
                progress = True
                while progress and n < maxn:
                    progress = False
                    for qi, (o0, nr, part, g, fi) in enumerate(fuse_q):
                        if part != pumped_parts.get(fi, 0):
                            continue   # keep per-fchunk part order
                        if XFIRST and part == 0:
                            lag = 1
                        elif fi >= len(FCHUNKS) - LATE_N:
                            lag = PUMP_LAG_LATE
                        else:
                            lag = PUMP_LAG
                        if group is not None and group < g + lag:
                            continue
                        fuse_q.pop(qi)
                        fuse_unit(o0, nr, part)
                        pumped_parts[fi] = part + 1
                        n += 1
                        progress = True
                        break

            fuse_emitted = 0
            x_emitted = 0
            gidx = 0
            for ci, (r0, nrc) in enumerate(CHUNKS):
                nb = nrc // 2   # rows per PSUM bank (uniform 2-bank tiles)
                for dd, d in enumerate(DILS):
                    gidx += 1
                    prod = ppool.tile([128, 9, 10, W], bf16, tag="pr")
                    # small chunks fit one PSUM bank: single matmul per tap
                    nbk = 2 if nrc * W * 4 > 2048 else 1
                    nb = nrc // nbk
                    for t in range(9):
                        di, dj = t // 3 - 1, t % 3 - 1
                        dt = dd * 9 + t
                        kp = kpool.tile([128, 2, 512], f32, tag="kp")
                        # each matmul writes within a single PSUM bank
                        for k in range(nbk):
                            nc.tensor.matmul(
                                kp[:, k, 0:nb * W],
                                wgd[dd][:, t * 128:(t + 1) * 128],
                                ysbc[ci][:, k * nb * W:(k + 1) * nb * W],
                                start=True, stop=True,
                            )
                        kv = kp[:, 0:nbk, 0:nb * W].rearrange(
                            "p b (r w) -> p b r w", w=W)
                        x0 = r0 + di * d + 5
                        xs = xpad[:, x0:x0 + nrc, 5 + dj * d:5 + dj * d + W
                                  ].rearrange("p (b r) w -> p b r w", r=nb)
                        pv = prod[:, t, 0:nrc, :].rearrange(
                            "p (b r) w -> p b r w", r=nb)
                        if t in ACT_TAPS:
                            ks = kspool.tile([128, 10, W], bf16, tag="ks")
                            ksv = ks[:, 0:nrc, :].rearrange(
                                "p (b r) w -> p b r w", r=nb)
                            nc.scalar.activation(
                                ksv, kv, ident, bias=gbsb[:, dt:dt + 1])
                            if t in POOL_MULT:
                                nc.gpsimd.tensor_tensor(pv, ksv, xs, mult)
                            else:
                                nc.vector.tensor_tensor(pv, ksv, xs, mult)
                        else:
                            nc.vector.scalar_tensor_tensor(
                                pv, kv, gbsb[:, dt:dt + 1], xs, add, mult)
                        if TAP_PUMP and t % TAP_PUMP == TAP_PUMP - 1:
                            pump_fuse(1, group=gidx)
                    # accumulation tree (bf16).  CCE accumulating DMAs are
                    # only correct up to ~2 planes (3840B/partition) per
                    # transfer, so level 0 is two independent 2-plane DMAs.
                    #   L0a: prod[0:2] += prod[4:6]  (DMA cce-add)
                    #   L0b: prod[2:4] += prod[6:8]  (DMA cce-add)
                    #   L1:  prod[0:2] += prod[2:4]  (DVE 2x)
                    #   L2:  prod[0] += prod[1]      (DVE 2x)
                    #   L3:  racc_d = prod[0] + prod[8]  (DVE 2x)
                    rv = racc[:, dd, r0:r0 + nrc, 1:1 + W]
                    if DMA_ACC:
                        nc.gpsimd.dma_start(out=prod[:, 0:2, 0:nrc, :],
                                            in_=prod[:, 4:6, 0:nrc, :],
                                            accum_op=add)
                        nc.gpsimd.dma_start(out=prod[:, 2:4, 0:nrc, :],
                                            in_=prod[:, 6:8, 0:nrc, :],
                                            accum_op=add)
                    else:
                        nc.vector.tensor_tensor(prod[:, 0:4, 0:nrc, :],
                                                prod[:, 0:4, 0:nrc, :],
                                                prod[:, 4:8, 0:nrc, :], add)
                    if DMA_L1:
                        nc.gpsimd.dma_start(out=prod[:, 0:2, 0:nrc, :],
                                            in_=prod[:, 2:4, 0:nrc, :],
                                            accum_op=add)
                    else:
                        nc.vector.tensor_tensor(prod[:, 0:2, 0:nrc, :],
                                                prod[:, 0:2, 0:nrc, :],
                                                prod[:, 2:4, 0:nrc, :], add)
                    nc.vector.tensor_tensor(prod[:, 0, 0:nrc, :],
                                            prod[:, 0, 0:nrc, :],
                                            prod[:, 1, 0:nrc, :], add)
                    nc.vector.tensor_tensor(rv, prod[:, 0, 0:nrc, :],
                                            prod[:, 8, 0:nrc, :], add)
                    # zero out-of-image halo rows (reference zero-pads cat):
                    # row 0 of block A when h0==0, row 25 of block B when
                    # h0==48 -- per-core masks keep the program SPMD-uniform
                    if r0 == 0:
                        nc.vector.tensor_scalar_mul(
                            racc[:, dd, 0, 1:1 + W], racc[:, dd, 0, 1:1 + W],
                            rmsb[:, 0:1])
                    elif r0 + nrc == BLK:
                        nc.vector.tensor_scalar_mul(
                            racc[:, dd, BLK - 1, 1:1 + W],
                            racc[:, dd, BLK - 1, 1:1 + W], rmsb[:, 1:2])
                    if XFIRST:
                        # queue part-0 (pure-x matmuls, no racc dep) of the
                        # next fchunk once its psum tile can be free: fchunk
                        # fi-2 fully pumped
                        while x_emitted < len(FCHUNKS):
                            if x_emitted >= 2 and pumped_parts.get(
                                    x_emitted - 2, 0) < FUSE_PARTS:
                                break
                            o0x, nrx = FCHUNKS[x_emitted]
                            fuse_q.append((o0x, nrx, 0, gidx, x_emitted))
                            x_emitted += 1
                    pump_fuse(PUMP_N, group=gidx)
                # queue fuse chunks whose racc rows are fully computed
                while fuse_emitted < len(FCHUNKS):
                    o0, nr = FCHUNKS[fuse_emitted]
                    if o0 + nr > r0 + nrc:
                        break
                    fuse_q.extend([(o0, nr, p, gidx, fuse_emitted)
                                   for p in range(1 if XFIRST else 0,
                                                  FUSE_PARTS)])
                    fuse_emitted += 1
            while fuse_emitted < len(FCHUNKS):
                o0, nr = FCHUNKS[fuse_emitted]
                fuse_q.extend([(o0, nr, p, 99, fuse_emitted)
                               for p in range(1 if XFIRST else 0,
                                              FUSE_PARTS)])
                fuse_emitted += 1
            if XFIRST:
                while x_emitted < len(FCHUNKS):
                    o0x, nrx = FCHUNKS[x_emitted]
                    fuse_q.append((o0x, nrx, 0, 99, x_emitted))
                    x_emitted += 1
            while fuse_q:
                pump_fuse(len(fuse_q))
    nc.finalize()
    return nc


def _prep_inputs(x, y, gen_w, gen_b, fuse_w, fuse_b):
    import ml_dtypes
    bf = ml_dtypes.bfloat16
    # generator weights: W_dt[c', c] = gen_w[d*576 + c*9 + t, c'],
    # block-diagonal over the two H-blocks.
    w3 = gen_w.reshape(3, 64, 9, 64).transpose(3, 0, 2, 1).reshape(64, 27, 64)
    wgh = np.zeros((128, 27, 128), np.float32)
    wgh[0:64, :, 0:64] = w3
    wgh[64:128, :, 64:128] = w3
    wgh = np.ascontiguousarray(wgh.reshape(128, 27 * 128)).astype(bf)
    gbh = gen_b.reshape(3, 64, 9).transpose(1, 0, 2).reshape(64, 27)
    gbh = np.ascontiguousarray(np.concatenate([gbh, gbh], 0))
    # fuse weights: [k, ij, g, o] block-diagonal
    f3 = fuse_w.transpose(1, 2, 3, 0).reshape(4, 64, 9, 64).transpose(1, 2, 0, 3)
    fwh = np.zeros((64, 9, 4, 128), np.float32)
    fwh[:, :, :, 0:64] = f3
    fwh2 = np.zeros((128, 9, 4, 128), np.float32)
    fwh2[0:64] = fwh
    fwh2[64:128, :, :, 64:128] = f3
    fwh = np.ascontiguousarray(fwh2.reshape(128, 9 * 4 * 128)).astype(bf)
    fbh = np.ascontiguousarray(np.concatenate([fuse_b, fuse_b])[:, None])
    xp = np.pad(x, ((0, 0), (0, 0), (6, 6), (5, 5))).astype(bf)
    yp = np.pad(y, ((0, 0), (0, 0), (1, 1), (0, 0))).astype(bf)
    maps = []
    for c in range(NCORES):
        n, half = c // 2, c % 2
        h0 = HH * half
        rmh = np.ones((128, 2), np.float32)
        if half == 0:
            rmh[0:64, 0] = 0.0     # block A row 0 = global row -1
        else:
            rmh[64:128, 1] = 0.0   # block B row 25 = global row 96
        maps.append({
            "xh": np.ascontiguousarray(xp[n, :, h0:h0 + 60, :]),
            "yh": np.ascontiguousarray(yp[n, :, h0:h0 + 50, :]),
            "wg": wgh, "gb": gbh, "fw": fwh, "fb": fbh, "rm": rmh,
        })
    return maps


def kernel(x, y, gen_w, gen_b, fuse_w, fuse_b):
    global _built
    from concourse.bass_utils import run_bass_kernel_spmd

    x = np.asarray(x, np.float32)
    y = np.asarray(y, np.float32)
    gen_w = np.asarray(gen_w, np.float32)
    gen_b = np.asarray(gen_b, np.float32)
    fuse_w = np.asarray(fuse_w, np.float32)
    fuse_b = np.asarray(fuse_b, np.float32)

    if _built is None:
        _built = _build()
    maps = _prep_inputs(x, y, gen_w, gen_b, fuse_w, fuse_b)
    res = run_bass_kernel_spmd(_built, maps, list(range(NCORES)))
    outf = np.empty((NB, C, H, W), np.float32)
    for c in range(NCORES):
        n, half = c // 2, c % 2
        outf[n, :, HH * half:HH * half + HH, :] = res.results[c]["out"]
    return outf

